# revision 45
# baseline (speedup 1.0000x reference)
"""Causal multi-head attention (B=2, S=2048, D=1024, H=16) on 8 NeuronCores.

Sharding: batch x head-group (2 x 4). Core c handles batch b = c // 4 and
head group g = c % 4 (4 heads = 256 model dims). Each core computes a full
[S, D] partial of the output projection for its batch; the host sums the 4
group partials per batch and adds the bias (the all-reduce of the
tensor-parallel split, done host-side as the unshard step).

Device pipeline per core (all matmuls bf16 with f32 PSUM accumulation):
  QT = Wq_g^T @ x_b^T        [256, 2048]   (lhsT=Wq chunk, rhs=xT chunk)
  KT = Wk_g^T @ x_b^T        [256, 2048]
  V  = x_b @ Wv_g            [2048, 256]
  per 512-wide q-block, per head pair (row-packed K=64 matmuls):
    ST tile = K_h @ Q_h^T    [128 k, 512 q] scores, transposed layout
    P = exp(0.125 * ST)      ACT, PSUM -> SBUF bf16, causal mask on diagonal
    ctx^T += V_h^T @ P       col-packed M=64 matmul pairs, f32 PSUM
    den   += ones^T @ P      col-packed ones-matmuls, aligned with ctx rows
    rb = exp(-ln(den))       ACT (reciprocal via ln/exp, same table set)
    ctxT_sbuf = ctx * rb     DVE, -> bf16
  out_partial = ctxT^T @ Wo_g   [2048, 1024] f32 -> DRAM
"""

import numpy as np
import ml_dtypes

B, S, D, H = 2, 2048, 1024, 16
HD = 64
G = 4            # head groups = tensor-parallel degree per batch
GD = D // G      # 256 dims per group
N_CORES = 8
BF16 = ml_dtypes.bfloat16

_CACHE = {}


def _build_nc():
    import concourse.mybir as mybir
    from concourse import bacc
    from concourse.tile import TileContext

    dt = mybir.dt
    f32 = dt.float32
    bf16 = dt.bfloat16
    AF = mybir.ActivationFunctionType

    # Pin all ACT functions to the natural_log_exp_and_others table set (it
    # holds Exp, Ln and Copy): with the default tables the fixpoint bounces
    # between exp_and_others and natural_log on every Ln/Exp alternation,
    # costing a ~1.3us table load each time. Stripping Exp/Ln from the other
    # sets (ids stay aligned with act_info.json) leaves one load total.
    from concourse import hw_specs

    AFT = mybir.ActivationFunctionType
    _orig_tables = hw_specs.get_activation_tables

    def _pinned_tables(module_arch):
        tables = dict(_orig_tables(module_arch))
        for name, fns in tables.items():
            if name != "natural_log_exp_and_others":
                tables[name] = fns - {AFT.Exp, AFT.Ln, AFT.Copy, AFT.Identity}
        return tables

    bacc.get_activation_tables = _pinned_tables

    nc = bacc.Bacc("TRN2")

    xT = nc.dram_tensor("xT", [D, S], bf16, kind="ExternalInput")
    wq = nc.dram_tensor("wq", [D, GD], bf16, kind="ExternalInput")
    wk = nc.dram_tensor("wk", [D, GD], bf16, kind="ExternalInput")
    wv = nc.dram_tensor("wv", [D, GD], bf16, kind="ExternalInput")
    wo = nc.dram_tensor("wo", [GD, D], bf16, kind="ExternalInput")
    masks = nc.dram_tensor("masks", [128, 1024], bf16, kind="ExternalInput")
    outp = nc.dram_tensor("out", [S, D], f32, kind="ExternalOutput")

    KC = D // 128   # 8 contraction chunks of 128
    NQ = S // 512   # 4 q blocks
    NR = S // 128   # 16 seq row tiles

    with TileContext(nc) as tc:
        with tc.tile_pool(name="const", bufs=1) as cpool:
            # xT chunk-by-chunk (completion order = issue order) so the
            # k-outer projection loop can start on chunk 0 right away; each
            # weight in one strided DMA (per-DMA overhead dominates)
            def load_w(nm, dram):
                t = cpool.tile([128, KC * GD], bf16, tag=nm, name=nm + "_t")
                nc.sync.dma_start(
                    out=t.rearrange("p (c g) -> p c g", c=KC),
                    in_=dram.rearrange("(c p) g -> p c g", p=128))
                return [t[:, k * GD:(k + 1) * GD] for k in range(KC)]

            HS = S // 2

            def load_xt(k, hf):
                t = cpool.tile([128, HS], bf16, tag=f"xT{k}_{hf}",
                               name=f"xt{k}_{hf}")
                nc.sync.dma_start(
                    out=t, in_=xT[128 * k:128 * (k + 1), HS * hf:HS * (hf + 1)])
                return t

            # c0 weight slivers + first xT half-chunk first: the first
            # matmuls start as soon as these three small DMAs land
            wqc0 = cpool.tile([128, GD], bf16, tag="wqc0")
            nc.sync.dma_start(out=wqc0, in_=wq[0:128, :])
            xt_a = [load_xt(0, 0)]
            wkc0 = cpool.tile([128, GD], bf16, tag="wkc0")
            nc.sync.dma_start(out=wkc0, in_=wk[0:128, :])
            xt_a.append(load_xt(1, 0))
            wq_rest = cpool.tile([128, (KC - 1) * GD], bf16, tag="wq")
            nc.sync.dma_start(
                out=wq_rest.rearrange("p (c g) -> p c g", c=KC - 1),
                in_=wq[128:, :].rearrange("(c p) g -> p c g", p=128))
            wk_rest = cpool.tile([128, (KC - 1) * GD], bf16, tag="wk")
            nc.sync.dma_start(
                out=wk_rest.rearrange("p (c g) -> p c g", c=KC - 1),
                in_=wk[128:, :].rearrange("(c p) g -> p c g", p=128))
            wq_sb = [wqc0] + [wq_rest[:, k * GD:(k + 1) * GD] for k in range(KC - 1)]
            wk_sb = [wkc0] + [wk_rest[:, k * GD:(k + 1) * GD] for k in range(KC - 1)]
            for k in range(2, KC):
                xt_a.append(load_xt(k, 0))
            xt_b = [load_xt(k, 1) for k in range(KC)]
            xt_half = (xt_a, xt_b)
            mask_sb = cpool.tile([128, 1024], bf16, tag="masks")
            nc.sync.dma_start(out=mask_sb, in_=masks[:, :])
            wv_sb = load_w("wv", wv)
            wo_sb = cpool.tile([128, 2 * D], bf16, tag="wo")
            nc.sync.dma_start(
                out=wo_sb.rearrange("p (c o) -> p c o", c=2),
                in_=wo.rearrange("(c p) o -> p c o", p=128))
            ones_sb = cpool.tile([128, 64], bf16, tag="ones")
            nc.vector.memset(ones_sb, 1.0)

            # persistent per-core intermediates
            qT_sb = cpool.tile([128, 2 * S], bf16, tag="qT")    # m-tile m at cols m*S
            kT_sb = cpool.tile([128, 2 * S], bf16, tag="kT")
            v_sb = []
            for jj in range(NQ):  # row tiles 4jj..4jj+3 at cols (r%4)*GD
                t = cpool.tile([128, 4 * GD], bf16, tag=f"v{jj}", name=f"v{jj}")
                v_sb.append(t)
            ctxT_sb = cpool.tile([128, 2 * S], bf16, tag="ctxT")  # pair p at cols p*S

            # One PSUM pool for the whole kernel (no pool-boundary WAR
            # stalls). 8 banks: sc0, sc1 ([128,1024] = 2 banks each), ctx0,
            # ctx1, den0, den1 ([128,512] each).
            #
            # Phase 1: QT/KT in two k-outer passes (n-halves) so PE starts on
            # xT chunk 0 as soon as its DMA lands. Per pass: 4 QT accs on the
            # ctx/den banks, 4 KT accs as halves of the two sc tiles.
            with tc.tile_pool(name="ps_att", bufs=1, space="PSUM") as ps_att, \
                 tc.tile_pool(name="sb_att", bufs=10) as sb_att, \
                 tc.tile_pool(name="sb_out", bufs=6) as sb_out:
                for half in range(2):
                    # qacc m=0 pair lives as halves of one sc2 tile; m=1 on
                    # the ctx/den banks; kaccs as halves of sc0/sc1
                    q0t = ps_att.tile([128, 1024], f32, tag="sc2", name="q0t")
                    qaccs = {}
                    for i, n in enumerate((2 * half, 2 * half + 1)):
                        qaccs[0, n] = q0t[:, 512 * i:512 * i + 512]
                        qaccs[1, n] = ps_att.tile(
                            [128, 512], f32, tag=("ctx", "den")[i],
                            name=f"qacc1{n}")
                    kaccs = {m: ps_att.tile([128, 1024], f32, tag=f"sc{m}",
                                            name=f"kacc{m}")
                             for m in range(2)}
                    xh = xt_half[half]
                    for k in range(KC):
                        for m in range(2):
                            for i, n in enumerate((2 * half, 2 * half + 1)):
                                nc.tensor.matmul(
                                    qaccs[m, n],
                                    lhsT=wq_sb[k][:, 128 * m:128 * m + 128],
                                    rhs=xh[k][:, 512 * i:512 * i + 512],
                                    start=(k == 0), stop=(k == KC - 1))
                        for m in range(2):
                            for i, n in enumerate((2 * half, 2 * half + 1)):
                                nc.tensor.matmul(
                                    kaccs[m][:, 512 * i:512 * i + 512],
                                    lhsT=wk_sb[k][:, 128 * m:128 * m + 128],
                                    rhs=xh[k][:, 512 * i:512 * i + 512],
                                    start=(k == 0), stop=(k == KC - 1))
                    for m in range(2):
                        for i, n in enumerate((2 * half, 2 * half + 1)):
                            nc.vector.tensor_copy(
                                qT_sb[:, m * S + 512 * n:m * S + 512 * n + 512],
                                qaccs[m, n])
                        nc.scalar.copy(
                            kT_sb[:, m * S + 1024 * half:m * S + 1024 * half + 1024],
                            kaccs[m])

                def emit_v(r):
                    # V projection row-tile: independent PE work that fills
                    # exp-wait gaps
                    vacc = ps_att.tile([128, GD], f32, name="vacc",
                                       tag="sc%d" % (r % 2))
                    for k in range(KC):
                        nc.tensor.matmul(
                            vacc,
                            lhsT=xt_half[r // 8][k][:, 128 * (r % 8):128 * (r % 8) + 128],
                            rhs=wv_sb[k],
                            start=(k == 0), stop=(k == KC - 1))
                    nc.vector.tensor_copy(
                        v_sb[r // 4][:, (r % 4) * GD:(r % 4 + 1) * GD], vacc)

                pending = []

                def emit_norm():
                    ctx_, den_, j_, pair_ = pending.pop(0)
                    lt = sb_att.tile([128, 512], f32, tag="ln", name="lt")
                    nc.scalar.activation(lt, den_, AF.Ln)
                    rb = sb_att.tile([128, 512], f32, tag="rb", name="rb")
                    nc.scalar.activation(rb, lt, AF.Exp, scale=-1.0)
                    nc.vector.tensor_mul(
                        ctxT_sb[:, pair_ * S + 512 * j_:pair_ * S + 512 * j_ + 512],
                        ctx_, rb)

                for j in range(NQ):
                    for r in range(4 * j, 4 * j + 4):
                        emit_v(r)
                    nkb = 4 * (j + 1)
                    for pair in range(2):
                        ctx = ps_att.tile([128, 512], f32, tag="ctx", name="ctx")
                        den = ps_att.tile([128, 512], f32, tag="den", name="den")
                        kbs = list(range(nkb - 1, -1, -1))
                        for ki, kb in enumerate(kbs):
                            # causal trim: on diagonal tiles only q-cols
                            # [128v, 512) can be unmasked (q >= k)
                            vv = kb - 4 * j
                            off = 128 * vv if vv > 0 else 0
                            w = 512 - off
                            sc = ps_att.tile([128, 1024], f32, tag="sc%d" % (kb % 3))
                            for hh in range(2):  # row-packed K=64 pair
                                nc.tensor.matmul(
                                    sc[:, 512 * hh + off:512 * hh + 512],
                                    lhsT=kT_sb[64 * hh:64 * hh + 64,
                                               pair * S + 128 * kb:pair * S + 128 * kb + 128],
                                    rhs=qT_sb[64 * hh:64 * hh + 64,
                                              pair * S + 512 * j + off:pair * S + 512 * j + 512],
                                    start=True, stop=True)
                            p_t = sb_att.tile([128, 1024], bf16, tag="p")
                            if off:
                                sc3 = sc.rearrange("p (h w) -> p h w", h=2)[:, :, off:]
                                pt3 = p_t.rearrange("p (h w) -> p h w", h=2)[:, :, off:]
                                nc.scalar.activation(pt3, sc3, AF.Exp, scale=0.125)
                            else:
                                nc.scalar.activation(p_t, sc, AF.Exp, scale=0.125)
                            if vv >= 0:
                                m3 = (mask_sb.rearrange("p (h w) -> p h w", h=2)
                                      [:, :, 0:w])
                                pt3 = p_t.rearrange("p (h w) -> p h w", h=2)[:, :, off:]
                                nc.vector.tensor_mul(pt3, pt3, m3)
                            st = (ki == 0)
                            sp = (ki == nkb - 1)
                            for hh in range(2):  # col-packed M=64 pairs
                                nc.tensor.matmul(
                                    ctx[64 * hh:64 * hh + 64, off:512],
                                    lhsT=v_sb[kb // 4][:, (kb % 4) * GD + 64 * (2 * pair + hh):
                                              (kb % 4) * GD + 64 * (2 * pair + hh) + 64],
                                    rhs=p_t[:, 512 * hh + off:512 * hh + 512],
                                    start=st, stop=sp)
                            for hh in range(2):
                                nc.tensor.matmul(
                                    den[64 * hh:64 * hh + 64, off:512],
                                    lhsT=ones_sb,
                                    rhs=p_t[:, 512 * hh + off:512 * hh + 512],
                                    start=st, stop=sp)
                            if ki == 1 and pending:
                                emit_norm()
                        pending.append((ctx, den, j, pair))
                    # ctxT(j) writes must be emitted before out-proj(j) reads
                    # them: Tile deps follow emission order
                    while pending:
                        emit_norm()
                    # output projection for the q rows finished in this j
                    for mr in range(4 * j, 4 * j + 4):
                        o_t = sb_out.tile([128, 1024], f32, tag="ot")
                        for nn in range(2):
                            acc = ps_att.tile([128, 512], f32, name="oacc",
                                              tag=("ctx", "den")[nn % 2])
                            for p in range(2):
                                nc.tensor.matmul(
                                    acc,
                                    lhsT=ctxT_sb[:, p * S + 128 * mr:p * S + 128 * mr + 128],
                                    rhs=wo_sb[:, p * D + 512 * nn:p * D + 512 * nn + 512],
                                    start=(p == 0), stop=(p == 1))
                            nc.vector.tensor_copy(o_t[:, 512 * nn:512 * nn + 512], acc)
                        nc.sync.dma_start(
                            out=outp[128 * mr:128 * mr + 128, :], in_=o_t)
    nc.compile()
    return nc


def _make_masks():
    # base causal pattern keep(t >= k), duplicated for the two packed heads;
    # diagonal tile variant v reads cols [0:512-128v] against P[:, 128v:512]
    kr = np.arange(128)[:, None]
    qc = np.arange(512)[None, :]
    m = np.zeros((128, 1024), BF16)
    mv = (qc >= kr).astype(BF16)
    m[:, 0:512] = mv
    m[:, 512:1024] = mv
    return m


def _prepare_in_maps(x, Wq, Wk, Wv, Wo):
    masks = _make_masks()
    in_maps = []
    xT = [np.ascontiguousarray(x[b].T).astype(BF16) for b in range(B)]
    for c in range(N_CORES):
        b, g = c // G, c % G
        sl = slice(GD * g, GD * (g + 1))
        in_maps.append({
            "xT": xT[b],
            "wq": np.ascontiguousarray(Wq[:, sl]).astype(BF16),
            "wk": np.ascontiguousarray(Wk[:, sl]).astype(BF16),
            "wv": np.ascontiguousarray(Wv[:, sl]).astype(BF16),
            "wo": np.ascontiguousarray(Wo[sl, :]).astype(BF16),
            "masks": masks,
        })
    return in_maps


def run_spmd(x, Wq, Wk, Wv, Wo, bo, **spmd_kwargs):
    from concourse.bass_utils import run_bass_kernel_spmd

    if "nc" not in _CACHE:
        _CACHE["nc"] = _build_nc()
    nc = _CACHE["nc"]
    in_maps = _prepare_in_maps(x, Wq, Wk, Wv, Wo)
    res = run_bass_kernel_spmd(nc, in_maps, core_ids=list(range(N_CORES)),
                               **spmd_kwargs)
    out = np.zeros((B, S, D), np.float32)
    for c in range(N_CORES):
        out[c // G] += res.results[c]["out"]
    out += np.asarray(bo, np.float32)[None, None, :]
    return out, res


def kernel(x, Wq, Wk, Wv, Wo, bo):
    x = np.asarray(x)
    out, _ = run_spmd(np.asarray(x, np.float32), np.asarray(Wq, np.float32),
                      np.asarray(Wk, np.float32), np.asarray(Wv, np.float32),
                      np.asarray(Wo, np.float32), np.asarray(bo, np.float32))
    return out
